# revision 22
# baseline (speedup 1.0000x reference)
"""Multi-head self-attention TRN2 kernel (8 NeuronCores, tensor-parallel on heads).

Sharding: core c owns heads (2c, 2c+1) for both batches. x is replicated
(pre-transposed on host to [C, B*T], bf16). Each core computes its two heads'
attention plus its slice of the output projection; the 8 partial outputs are
summed on the host (out_b and the v-bias fold added once).

Precision plan:
  - QKV projection in bf16 (x, w bf16; psum fp32). q gets its bias on DVE;
    k-bias is dropped (softmax shift-invariant); v-bias is folded into the
    host-side output bias (sum_s attn = 1 => + out_w @ bv).
  - q, k stay bf16 in SBUF as [128 = 2 heads x 64 dims, token]. Scores for
    the two heads run as K=64 row-tiled bf16 matmuls (tile_position (0,0) /
    (64,0) auto-derived from base partitions) which co-execute in the PE
    array's upper/lower row halves (~2x vs serial, HW-measured).
  - exp on ScalarE (psum->sbuf bf16), AV + projections bf16; y output fp16.

Per-core dataflow:
  - Scores are computed transposed (scoresT[ts, tq] = k . q) so the softmax
    denominator is recovered by appending a ones-column to V in the attn @ V
    matmul (contraction over ts = partitions). No max-subtraction: |scores/8|
    < ~3 for this problem's distributions, exp is safe in fp32.
  - vT is flipped to natural [token, feature] layout with DMA-xbar transposes.

Scheduling: eight attention sections (unit x tq-quarter, 512 wide). The
512-wide windows make every psum tile one bank: 6 score slots (3 window
pairs: one being exp'd, one ready, one being written two slots ahead) + 2
AV-accumulator banks fill PSUM exactly. The deep score rotation decouples
PE from ScalarE: qk(g+2) never waits on an exp read, exp(g) was satisfied
two slots ago. Deferred projection / v-transpose / output-projection items
drain inside the stream with per-slot PE budgets; ScalarE's 256-exp stream
(~103 us) hides under the ~140 us PE stream.
"""

import os
import sys

sys.path.insert(0, "/opt/trn_rl_repo")

import numpy as np
import ml_dtypes
from contextlib import ExitStack

import concourse.bass as bass
import concourse.bacc as bacc
import concourse.mybir as mybir
import concourse.tile as tile

F32 = mybir.dt.float32
F32R = mybir.dt.float32r
BF16 = mybir.dt.bfloat16
F16 = mybir.dt.float16

B, T, C, H, DK = 2, 2048, 1024, 16, 64
NCORE = 8
HPC = H // NCORE            # heads per core = 2
FQKV = 3 * HPC * DK         # 384 projection features per core
BT = B * T                  # 4096 tokens
KP = C // 128               # 8 contraction passes
TCH = 1024                  # token chunk for projection matmuls/DMA
NCHUNK = BT // TCH          # 4
TS_TILES = T // 128         # 16 key tiles per batch
HALF = 512                  # tq span per attention section

_CACHE = {}


def _emit(ctx, tc, xT, wq, bq, wo, onin, y):
    nc = tc.nc
    from collections import deque
    Exp = mybir.ActivationFunctionType.Exp
    Add = mybir.AluOpType.add

    wpool = ctx.enter_context(tc.tile_pool(name="w", bufs=1))
    xpool = ctx.enter_context(tc.tile_pool(name="x", bufs=16))
    vapool = ctx.enter_context(tc.tile_pool(name="va", bufs=2))
    aupool = ctx.enter_context(tc.tile_pool(name="au", bufs=8))
    aopool = ctx.enter_context(tc.tile_pool(name="ao", bufs=2))
    ypool = ctx.enter_context(tc.tile_pool(name="ysb", bufs=4))
    mpool = ctx.enter_context(tc.tile_pool(name="misc", bufs=2))
    scpool = ctx.enter_context(tc.tile_pool(name="sc", bufs=6, space="PSUM"))
    opool = ctx.enter_context(tc.tile_pool(name="po", bufs=2, space="PSUM"))

    # ---- constants / weights (x chunk 0 + w first; cold tensors after) ----
    wq_r = wq.rearrange("(n p) f -> p n f", p=128)
    w_sb = wpool.tile([128, KP, FQKV], BF16)
    nc.gpsimd.dma_start(out=w_sb, in_=wq_r[:, :, :])
    b_sb = wpool.tile([128, 1], F32)
    nc.sync.dma_start(out=b_sb, in_=bq.rearrange("(t p) -> p t", p=128))

    # ACT exp-table preload: a tiny exp right at kernel start pulls the
    # ~2.7us ACT_TABLE_LOAD off the exp-stream critical path.
    actw = mpool.tile([128, 8], F32, tag="aw", name="aw", bufs=1)
    nc.vector.memset(actw, 0)
    acto = mpool.tile([128, 8], BF16, tag="aw2", name="aw2", bufs=1)
    nc.scalar.activation(acto, actw, Exp, scale=1.0)

    # q/k bf16, feature-major: partition = head h * 64 + dim, free = token
    q_sb = wpool.tile([128, BT], BF16)
    k_sb = wpool.tile([128, BT], BF16)
    # v feature-major bf16 (transposed later per ts-tile)
    v_sb = wpool.tile([128, BT], BF16)

    # ---- helpers ----
    def xdma_chunk(chunk):
        xts = []
        for p in range(KP):
            eng = nc.sync if p % 2 == 0 else nc.gpsimd
            xt = xpool.tile([128, TCH], BF16, name=f"xt{chunk}_{p}", tag="xt")
            eng.dma_start(
                out=xt,
                in_=xT[p * 128:(p + 1) * 128, chunk * TCH:(chunk + 1) * TCH],
            )
            xts.append(xt)
        return xts

    def _proj_finish(chunk, f, th, ps):
        """Convert the 8-pass psum to bf16 and store into q/k/v SBUF."""
        lo = chunk * TCH + th * 512
        tsl = slice(lo, lo + 512)
        dst = (q_sb, k_sb, v_sb)[f]
        if f == 0:
            nc.vector.tensor_scalar_add(dst[:, tsl], ps, b_sb)
        else:
            nc.vector.tensor_copy(dst[:, tsl], ps)

    def proj_pieces(chunk, f, xts_holder):
        """One 2-phase item per 512-token half: passes 0-3 emitted at the
        end of one stream slot (psum alloc), passes 4-7 + convert early in
        the next slot."""
        items = []
        for th in range(2):
            st = {}

            def phaseA(f=f, th=th, st=st):
                nsl = slice(th * 512, (th + 1) * 512)
                ps = scpool.tile([128, 512], F32, tag="sc",
                                 name=f"pp{chunk}_{f}_{th}")
                for p in range(4):
                    nc.tensor.matmul(
                        ps, w_sb[:, p, f * 128:(f + 1) * 128],
                        xts_holder["x"][p][:, nsl],
                        start=(p == 0), stop=False,
                    )
                st["ps"] = ps

            def phaseB(f=f, th=th, st=st):
                nsl = slice(th * 512, (th + 1) * 512)
                ps = st.pop("ps")
                for p in range(4, KP):
                    nc.tensor.matmul(
                        ps, w_sb[:, p, f * 128:(f + 1) * 128],
                        xts_holder["x"][p][:, nsl],
                        start=False, stop=(p == KP - 1),
                    )
                _proj_finish(chunk, f, th, ps)
            items.append((phaseA, phaseB))
        return items

    VAW = 192   # per-ts-tile va row: [h0 d0:64 | ones | pad | h1 d0:64 @96 | ones]
    def alloc_va(u):
        va = vapool.tile([128, TS_TILES, VAW], BF16, name=f"va{u}", tag="va")
        ones_bc = bass.AP(
            tensor=ones_sb.tensor,
            offset=ones_sb.offset,
            ap=[ones_sb.ap[0], [0, TS_TILES], [0, 1]],
        )
        nc.vector.tensor_copy(va[:, :, DK:DK + 1], ones_bc)
        nc.vector.tensor_copy(va[:, :, 96 + DK:96 + DK + 1], ones_bc)
        return va

    def transp_item(u, va, i):
        # XBAR DMA transpose: destinations kept 32-element aligned (head
        # slots at 0 and 96; i-stride 192) so full xbar tiles never touch
        # the ones columns.
        def go():
            tsl = slice(u * T + i * 128, u * T + (i + 1) * 128)
            for h in range(HPC):
                nc.sync.dma_start_transpose(
                    out=va[:, i, 96 * h:96 * h + DK],
                    in_=v_sb[h * DK:(h + 1) * DK, tsl],
                )
        return go

    post = {"on": False, "n": 0}

    def yp_half(u, ao, t0, n, tag):
        def go():
            yp = scpool.tile([128, 512], F32, tag="sc", name=f"yp{tag}_{n}")
            nc.tensor.matmul(
                yp, ao[:, t0:t0 + 128], wo_sb[:, n * 512:(n + 1) * 512],
                start=True, stop=True,
            )
            ys = ypool.tile([128, 512], F16, name=f"ys{tag}_{n}", tag="ys")
            # post-loop (exp stream done): alternate the psum extraction
            # between ScalarE and DVE so the tail drains at 2x
            post["n"] += 1
            if post["on"] and post["n"] % 2 == 0:
                nc.scalar.copy(ys, yp)
            else:
                nc.vector.tensor_copy(ys, yp)
            eng = nc.sync if (t0 // 128 + n) % 2 == 0 else nc.gpsimd
            eng.dma_start(
                out=y[u * T + t0:u * T + t0 + 128, n * 512:(n + 1) * 512],
                in_=ys,
            )
        return go

    # ---- deadline queue (dq) + filler queue (fq) ----
    # dq items: (deadline (si, i), PE-cost us, fn). Dependent items always
    # have deadline >= their producer's, so running all due items in queue
    # order is dependency-safe even when deadlines aren't monotonic.
    # fq: no-deadline fillers (yp halves).
    dq = []
    fq = deque()
    BUDGET = 0.45

    pend_b = []

    def _run_item(fn, forced):
        if isinstance(fn, tuple):
            fa, fb = fn
            fa()
            if forced:
                fb()
            else:
                pend_b.append(fb)
        else:
            fn()

    def drain(slot):
        budget = 0.2 if slot < (0, 3) else BUDGET
        i = 0
        while i < len(dq):
            if dq[i][0] <= slot:
                _, c, fn = dq.pop(i)
                _run_item(fn, True)
                budget -= c
            else:
                i += 1
        while budget > 0:
            if dq:
                d, c, fn = dq[0]
                if c <= budget + 0.6:
                    dq.pop(0)
                    _run_item(fn, False)
                    budget -= c
                    continue
            if fq:
                c, fn = fq[0]
                if c <= budget + 0.3:
                    fq.popleft(); fn(); budget -= c
                    continue
            break

    # ---- scores: one 512-col MM per head per key-tile, K=64 row-tiled ----
    sections = [(u, qtr) for u in range(2) for qtr in range(4)]
    NT = len(sections) * TS_TILES   # 128 stream slots

    def qk_pair(g):
        si, i = g // TS_TILES, g % TS_TILES
        u, qtr = sections[si]
        q0 = u * T + qtr * HALF
        ksl = slice(u * T + i * 128, u * T + (i + 1) * 128)
        ps = []
        for h in range(HPC):
            hp = slice(DK * h, DK * (h + 1))
            ps_ = scpool.tile([128, HALF], F32, tag="sc", name=f"s{si}_{i}_{h}")
            nc.tensor.matmul(
                ps_, k_sb[hp, ksl], q_sb[hp, q0:q0 + HALF],
                start=True, stop=True,
            )
            ps.append(ps_)
        return ps

    # ---- fill: chunk 0 q,k; then qk(0), qk(1) so the exp stream starts
    # ASAP with two windows banked; v c0 + first transposes ride in the
    # exp shadow ----
    x0 = xdma_chunk(0)
    # cold tensors after the critical x/w loads
    wo_sb = wpool.tile([128, C], BF16)
    nc.sync.dma_start(out=wo_sb, in_=wo[:, :])
    ones_sb = wpool.tile([128, 1], F32R)
    nc.sync.dma_start(out=ones_sb, in_=onin[:, :])
    # PE pstate warmup on a zeroed scratch tile (no DMA dependency): ~3.5us
    # of dummy matmuls while x streams in, so the projection runs at full
    # clock (the ramp needs 3us of continuous PE busy)
    wz = mpool.tile([128, 512], F32, tag="wz", name="wz", bufs=1)
    nc.vector.memset(wz, 0)
    wzr = wz.bitcast(F32R)
    for wi in range(9):
        wu = opool.tile([128, 512], F32, tag="po", name=f"wu{wi}")
        nc.tensor.matmul(wu, wzr[:, 0:128], wzr, start=True, stop=True)
    # q,k projection, x-tile-interleaved (PE keeps pace with the x DMA)
    for th in range(2):
        nsl = slice(th * 512, (th + 1) * 512)
        psq = scpool.tile([128, 512], F32, tag="sc", name=f"pp0_0_{th}")
        psk = scpool.tile([128, 512], F32, tag="sc", name=f"pp0_1_{th}")
        for p in range(KP):
            for f, ps in ((0, psq), (1, psk)):
                nc.tensor.matmul(
                    ps, w_sb[:, p, f * 128:(f + 1) * 128], x0[p][:, nsl],
                    start=(p == 0), stop=(p == KP - 1),
                )
        _proj_finish(0, 0, th, psq)
        _proj_finish(0, 1, th, psk)
    pss_ring = {0: qk_pair(0)}
    vas = {0: alloc_va(0)}
    h0_ = {"x": x0}
    v0_pieces = proj_pieces(0, 2, h0_)
    v0_pieces[0][0](); v0_pieces[0][1]()   # v c0 tokens 0:512
    pss_ring[1] = qk_pair(1)
    for i in range(0, 4):
        transp_item(0, vas[0], i)()
    v0_pieces[1][0](); v0_pieces[1][1]()   # v c0 tokens 512:1024 (exp shadow)
    for i in range(4, 8):
        transp_item(0, vas[0], i)()
    h1 = {"x": xdma_chunk(1)}

    PP, PT = 1.7, 0.05
    for n_, it in enumerate(proj_pieces(1, 1, h1)):   # k c1: qk((0,8)) @ slot 6
        dq.append(((0, 3 + 2 * n_), PP, it))
    for n_, it in enumerate(proj_pieces(1, 2, h1)):   # v c1: t(0, 8..15)
        dq.append(((0, 6 + n_ // 2), PP, it))
    for i in range(8, TS_TILES):
        dq.append(((0, i), PT, transp_item(0, vas[0], i)))
    for n_, it in enumerate(proj_pieces(1, 0, h1)):   # q c1: sections 2,3
        dq.append(((1, 4 + 2 * n_), PP, it))
    h2 = {}
    dq.append(((1, 12), 0.1, lambda: h2.update(x=xdma_chunk(2))))

    # ---- flat 128-slot stream: exp(g) | qk(g+2) | AV(g-1) | drain ----
    aos, pos = {}, {}
    h3 = {}

    def emit_av(g, aus):
        si, i = g // TS_TILES, g % TS_TILES
        u, qtr = sections[si]
        if i == 0:
            pos[si] = [
                opool.tile([DK + 1, HALF], F32, tag="po", name=f"po{si}_{h}")
                for h in range(HPC)
            ]
        va = vas[u]
        for h in range(HPC):
            vsl = slice(96 * h, 96 * h + DK + 1)
            nc.tensor.matmul(
                pos[si][h], va[:, i, vsl], aus[h],
                start=(i == 0), stop=(i == TS_TILES - 1),
            )

    def emit_norm(si):
        u, qtr = sections[si]
        if u not in aos:
            aos[u] = aopool.tile([128, T], BF16, name=f"ao{u}", tag="ao")
        ao, po = aos[u], pos[si]
        ncb = 4 if si == len(sections) - 1 else 1
        w_ = HALF // ncb
        for cb in range(ncb):
            for h in range(HPC):
                csl = slice(cb * w_, (cb + 1) * w_)
                r1 = mpool.tile([1, 512], F32, tag="r1",
                                name=f"r1{si}_{h}{cb}")
                nc.vector.reciprocal(r1[:, 0:w_], po[h][DK:DK + 1, csl])
                rb = mpool.tile([DK, 512], F32, tag="rb",
                                name=f"rb{si}_{h}{cb}")
                nc.gpsimd.partition_broadcast(rb[:, 0:w_], r1[:, 0:w_])
                c0 = qtr * HALF + cb * w_
                nc.vector.tensor_mul(
                    ao[h * DK:(h + 1) * DK, c0:c0 + w_],
                    po[h][0:DK, csl],
                    rb[:, 0:w_],
                )
        for m in range(HALF // 128):
            for n in range(C // 512):
                fq.append((0.72, yp_half(u, ao, qtr * HALF + m * 128, n,
                                         f"{si}_{m}")))
        # deferred projections for the following sections
        if si == 1:
            for n_, it in enumerate(proj_pieces(2, 1, h2)):   # k c2
                dq.append(((2, 8 + 2 * n_), PP, it))
            for n_, it in enumerate(proj_pieces(2, 2, h2)):   # v c2
                dq.append(((3, 2 * n_), PP, it))
            vas[1] = alloc_va(1)
            for i_ in range(8):
                dq.append(((3, 4 + i_), PT, transp_item(1, vas[1], i_)))
            for n_, it in enumerate(proj_pieces(2, 0, h2)):   # q c2
                dq.append(((3, 8 + 2 * n_), PP, it))
            dq.append(((3, 8), 0.1, lambda: h3.update(x=xdma_chunk(3))))
        elif si == 3:
            for n_, it in enumerate(proj_pieces(3, 1, h3)):   # k c3
                dq.append(((4, 1 + 2 * n_), PP, it))
            for n_, it in enumerate(proj_pieces(3, 2, h3)):   # v c3
                dq.append(((4, 4 + 2 * n_), PP, it))
            for i_ in range(8, TS_TILES):
                dq.append(((4, i_), PT, transp_item(1, vas[1], i_)))
            for n_, it in enumerate(proj_pieces(3, 0, h3)):   # q c3
                dq.append(((5, 4 + 2 * n_), PP, it))

    prev = None
    for g in range(NT):
        si, i = g // TS_TILES, g % TS_TILES
        pss = pss_ring.pop(g)
        aus = []
        for h in range(HPC):
            au = aupool.tile([128, HALF], BF16, name=f"au{si}_{i}_{h}",
                             tag="au")
            nc.scalar.activation(au, pss[h], Exp, scale=0.125)
            aus.append(au)
        if g + 2 < NT:
            pss_ring[g + 2] = qk_pair(g + 2)
        for fb in pend_b:
            fb()
        pend_b.clear()
        if prev is not None:
            emit_av(g - 1, prev)
            if (g - 1) % TS_TILES == TS_TILES - 1:
                emit_norm(si - 1)
        drain((si, i))
        prev = aus

    emit_av(NT - 1, prev)
    emit_norm(len(sections) - 1)
    post["on"] = True
    for fb in pend_b:
        fb()
    pend_b.clear()
    while dq:
        _run_item(dq.pop(0)[2], True)
    while fq:
        fq.popleft()[1]()


def _build(repeat=1):
    key = ("nc", repeat)
    if key in _CACHE:
        return _CACHE[key]
    nc = bacc.Bacc("TRN2", target_bir_lowering=False)
    xT = nc.dram_tensor("xT", [C, BT], BF16, kind="ExternalInput")
    wq = nc.dram_tensor("wqkvT", [C, FQKV], BF16, kind="ExternalInput")
    bq = nc.dram_tensor("bq", [128], F32, kind="ExternalInput")
    wo = nc.dram_tensor("woT", [HPC * DK, C], BF16, kind="ExternalInput")
    onin = nc.dram_tensor("ones", [128, 1], F32R, kind="ExternalInput")
    y = nc.dram_tensor("y", [BT, C], F16, kind="ExternalOutput")
    with tile.TileContext(nc) as tc:
        for _ in range(repeat):
            with ExitStack() as ctx:
                _emit(ctx, tc, xT[:], wq[:], bq[:], wo[:], onin[:], y[:])
    nc.compile()
    nc.finalize()
    _CACHE[key] = nc
    return nc


def make_in_maps(x, qkv_w, qkv_b, out_w):
    """Host-side sharding: returns the 8 per-core input maps."""
    x = np.asarray(x, dtype=np.float32)
    qkv_w = np.asarray(qkv_w, dtype=np.float32)
    qkv_b = np.asarray(qkv_b, dtype=np.float32)
    out_w = np.asarray(out_w, dtype=np.float32)
    xTh = np.ascontiguousarray(x.reshape(BT, C).T).astype(ml_dtypes.bfloat16)
    in_maps = []
    for c in range(NCORE):
        r = slice(128 * c, 128 * (c + 1))
        wsl = np.concatenate([qkv_w[r], qkv_w[C:][r], qkv_w[2 * C:][r]], axis=0)
        in_maps.append(
            {
                "xT": xTh,
                "wqkvT": np.ascontiguousarray(wsl.T).astype(ml_dtypes.bfloat16),
                "bq": np.ascontiguousarray(qkv_b[r]),
                "woT": np.ascontiguousarray(out_w[:, r].T).astype(ml_dtypes.bfloat16),
                "ones": np.ones((128, 1), dtype=np.float32),
            }
        )
    return in_maps


# ---------------- cached PJRT runner (avoids per-call retracing) ----------------

def _make_runner(nc, n_cores=NCORE):
    import jax
    from jax.sharding import Mesh, PartitionSpec
    from jax.experimental.shard_map import shard_map
    from concourse import bass2jax

    bass2jax.install_neuronx_cc_hook()
    partition_name = (
        nc.partition_id_tensor.name if nc.partition_id_tensor else None
    )
    in_names, out_names, out_avals = [], [], []
    for alloc in nc.m.functions[0].allocations:
        if not isinstance(alloc, mybir.MemoryLocationSet):
            continue
        name = alloc.memorylocations[0].name
        if alloc.kind == "ExternalInput":
            if name != partition_name:
                in_names.append(name)
        elif alloc.kind == "ExternalOutput":
            out_avals.append(
                jax.core.ShapedArray(
                    tuple(alloc.tensor_shape), mybir.dt.np(alloc.dtype)
                )
            )
            out_names.append(name)

    all_in_names = list(in_names) + list(out_names)
    if partition_name is not None:
        all_in_names.append(partition_name)

    def _body(*args):
        operands = list(args)
        if partition_name is not None:
            operands.append(bass2jax.partition_id_tensor())
        outs = bass2jax._bass_exec_p.bind(
            *operands,
            out_avals=tuple(out_avals),
            in_names=tuple(all_in_names),
            out_names=tuple(out_names),
            lowering_input_output_aliases=(),
            sim_require_finite=True,
            sim_require_nnan=True,
            nc=nc,
        )
        return tuple(outs)

    devices = jax.devices()[:n_cores]
    mesh = Mesh(np.asarray(devices), ("core",))
    in_specs = (PartitionSpec("core"),) * (len(in_names) + len(out_names))
    out_specs = (PartitionSpec("core"),) * len(out_names)
    fn = jax.jit(
        shard_map(_body, mesh=mesh, in_specs=in_specs, out_specs=out_specs,
                  check_rep=False)
    )
    return fn, in_names, out_avals, mesh


def _get_runner(repeat=1):
    key = ("runner", repeat)
    if key not in _CACHE:
        _CACHE[key] = _make_runner(_build(repeat))
    return _CACHE[key]


def _run(in_maps, repeat=1):
    import jax
    from jax.sharding import NamedSharding, PartitionSpec

    fn, in_names, out_avals, mesh = _get_runner(repeat)
    sh = NamedSharding(mesh, PartitionSpec("core"))
    dev_ins = []
    for name in in_names:
        big = np.concatenate([m[name] for m in in_maps], axis=0)
        dev_ins.append(jax.device_put(big, sh))
    for av in out_avals:
        big = np.zeros((av.shape[0] * NCORE,) + tuple(av.shape[1:]), av.dtype)
        dev_ins.append(jax.device_put(big, sh))
    out = fn(*dev_ins)
    jax.block_until_ready(out)
    return np.asarray(out[0])


def kernel(x, qkv_w, qkv_b, out_w, out_b):
    x = np.asarray(x, dtype=np.float32)
    qkv_w = np.asarray(qkv_w, dtype=np.float32)
    qkv_b = np.asarray(qkv_b, dtype=np.float32)
    out_w = np.asarray(out_w, dtype=np.float32)
    out_b = np.asarray(out_b, dtype=np.float32)

    in_maps = make_in_maps(x, qkv_w, qkv_b, out_w)
    ybig = _run(in_maps)                      # [NCORE*BT, C] fp16
    parts = ybig.reshape(NCORE, BT, C)
    # v-bias is folded here: sum_s attn = 1  =>  + out_w @ bv; k-bias dropped
    # (softmax shift-invariance).
    bv = qkv_b[2 * C:]
    bias = out_b.astype(np.float64) + out_w.astype(np.float64) @ bv.astype(np.float64)
    out = parts.astype(np.float64).sum(axis=0) + bias
    return out.reshape(B, T, C).astype(np.float32)
